# revision 1
# baseline (speedup 1.0000x reference)
"""Trainium2 Bass kernel for nn_CovarianceEstimator.

Computes, for y [B=16, R=1, A=16, T=14, S=1024] complex (given as separate
real/imag f32 tensors):
  - gather P=1024 pilot positions (sym_p, sc_p) from estimation_indices
  - per-position A x A outer products sig_p sig_p^H
  - unsorted-segment-mean over subcarrier ids sc_p
  - nearest-neighbor expand via closest_subcarrier to all S subcarriers
  - broadcast over T symbols
Output: [B, R, T, S, A, A] complex64.

Sharding: data-parallel over batch; 2 batches per core on 8 cores.

Two device-program builders:
  * fast path  - used when the index tensors match the PilotPattern structure
                 (meshgrid of 2 symbols x every-2nd-subcarrier, closest = even
                 floor).  Pure DVE + DMA, exact f32 math.
  * generic    - any estimation_indices / closest_subcarrier.  Host folds the
                 whole segment-mean + NN-gather into one dense [S, P] weight
                 matrix applied on the tensor engine.
"""

import numpy as np

B, R, A, T, S = 16, 1, 16, 14, 1024
P_EST = 1024          # number of (sym, sc) estimation positions
N_CORES = 8
B_LOC = B // N_CORES  # 2 batches per core
AA2 = A * A * 2       # interleaved (re, im) row payload per subcarrier

_cache = {}


def _fast_path_info(est, closest):
    """Return (sym0, sym1) if indices match the pilot-pattern structure:
    est == meshgrid([sym0, sym1], arange(0, S, 2)) row-major and
    closest == 2*(arange(S)//2).  Else None."""
    if est.shape != (P_EST, 2) or closest.shape != (S,):
        return None
    sc = np.arange(0, S, 2, dtype=est.dtype)
    if not np.array_equal(est[: S // 2, 1], sc):
        return None
    if not np.array_equal(est[S // 2 :, 1], sc):
        return None
    sym0 = int(est[0, 0])
    sym1 = int(est[S // 2, 0])
    if not (0 <= sym0 < T and 0 <= sym1 < T):
        return None
    if not np.all(est[: S // 2, 0] == sym0):
        return None
    if not np.all(est[S // 2 :, 0] == sym1):
        return None
    if not np.array_equal(closest, (2 * (np.arange(S) // 2)).astype(closest.dtype)):
        return None
    return sym0, sym1


def _build_fast(sym0, sym1):
    """DVE-only program.  Per batch:
      - DMA the two pilot-symbol slabs y[b,:,sym,:] into SBUF laid out
        [q, a, k] with subcarrier s = q*8 + k  (contiguous innermost runs).
      - strided on-chip copy selects even subcarriers: sig_h[q, m, a] for
        pair index s' = q*4 + m  (sc = 2*s'), scaled by sqrt(1/2) so every
        product carries the segment-mean 1/2.
      - DVE outer products + pair sums -> f[q, m, i*A+j, re/im]
      - duplicate rows (nearest-neighbor expand) into fd so each partition
        holds output rows s = q*8 .. q*8+7 contiguously, then one plain
        [128, 4096] DMA per (b, t)."""
    import concourse.bacc as bacc
    import concourse.mybir as mybir
    from concourse.tile import TileContext

    f32 = mybir.dt.float32
    nc = bacc.Bacc(trn_type="TRN2", target_bir_lowering=False)
    yr = nc.declare_dram_parameter("yr", [B_LOC, A, T, S], f32, isOutput=False)
    yi = nc.declare_dram_parameter("yi", [B_LOC, A, T, S], f32, isOutput=False)
    out = nc.declare_dram_parameter("out", [B_LOC, T, S, AA2], f32, isOutput=True)

    KS = S // 128  # 8 subcarriers per partition
    M = KS // 2    # 4 subcarrier-pairs per partition

    with TileContext(nc) as tc:
        with (
            tc.tile_pool(name="slab", bufs=2) as slabp,
            tc.tile_pool(name="sig", bufs=2) as sigp,
            tc.tile_pool(name="g", bufs=2) as gp,
            tc.tile_pool(name="f", bufs=2) as fp,
        ):
            for b in range(B_LOC):
                # per-pilot-symbol slab loads: [q, a, k] with s = q*8+k.
                # Small (64KB) so the serial prefix before compute is short;
                # spread across the two DGE paths.
                sr = sigp.tile([128, 2, M, A], f32, tag="sr")  # [q, h, m, a]
                si = sigp.tile([128, 2, M, A], f32, tag="si")
                for part, (ysrc, dst, eng) in enumerate(
                    ((yr, sr, nc.scalar), (yi, si, nc.gpsimd))
                ):
                    for h, sym in enumerate((sym0, sym1)):
                        slab = slabp.tile([128, A, KS], f32, tag=f"slab{part}{h}")
                        eng.dma_start(
                            out=slab[:],
                            in_=ysrc[b, :, sym, :].rearrange(
                                "a (q k) -> q a k", q=128, k=KS
                            ),
                        )
                        # select even subcarriers, transpose (a,k)->(m,a),
                        # scale by sqrt(1/2) for the segment mean
                        nc.vector.tensor_scalar_mul(
                            dst[:, h],
                            slab[:, :, 0:KS:2].transpose([0, 2, 1]),
                            0.7071067811865476,
                        )

                HM = 2 * M  # merged (h, m) dim

                def vi(x):  # varies over i, broadcast over j; h merged in
                    return (
                        x[:]
                        .rearrange("q h m a -> q (h m) a")[:, :, :, None]
                        .to_broadcast([128, HM, A, A])
                    )

                def vj(x):  # broadcast over i, varies over j
                    return (
                        x[:]
                        .rearrange("q h m a -> q (h m) a")[:, :, None, :]
                        .to_broadcast([128, HM, A, A])
                    )

                # fd[q, m, e, i*A+j, re/im]: output rows s = q*8 + m*2 + e
                fd = fp.tile([128, M, 2, A * A, 2], f32, tag="fd")
                u0 = gp.tile([128, HM, A, A], f32, tag="u0")
                u1 = gp.tile([128, HM, A, A], f32, tag="u1")
                v0 = gp.tile([128, M, A, A], f32, tag="v0")
                v1 = gp.tile([128, M, A, A], f32, tag="v1")
                # real: sum_h SrSr + SiSi, written to both e slots
                nc.vector.tensor_mul(u0[:], vi(sr), vj(sr))
                nc.vector.tensor_mul(u1[:], vi(si), vj(si))
                nc.vector.tensor_add(v0[:], u0[:, :M], u0[:, M:])
                nc.vector.tensor_add(v1[:], u1[:, :M], u1[:, M:])
                nc.vector.tensor_add(fd[:, :, 0, :, 0], v0[:], v1[:])
                nc.vector.tensor_add(fd[:, :, 1, :, 0], v0[:], v1[:])
                # imag: sum_h SiSr - SrSi
                nc.vector.tensor_mul(u0[:], vi(si), vj(sr))
                nc.vector.tensor_mul(u1[:], vi(sr), vj(si))
                nc.vector.tensor_add(v0[:], u0[:, :M], u0[:, M:])
                nc.vector.tensor_add(v1[:], u1[:, :M], u1[:, M:])
                nc.vector.tensor_sub(fd[:, :, 0, :, 1], v0[:], v1[:])
                nc.vector.tensor_sub(fd[:, :, 1, :, 1], v0[:], v1[:])

                # --- output rows s = q*8 + (m*2+e), contiguous per partition.
                # One fused DMA per batch covers all T symbol copies via a
                # stride-0 t dim on the SBUF source.  The walrus DIRECT2D DMA
                # form accepts only ONE sync wait; with slab loads on SWDGE
                # lanes, each batch's single output DMA lands on a fresh HWDGE
                # lane and needs only the fd-ready wait.
                src = (
                    fd[:]
                    .rearrange("q m e c ri -> q (m e c ri)")[:, None, :]
                    .to_broadcast([128, T, M * 2 * A * A * 2])
                )
                dst = out[b].rearrange("t (q k) c -> q t (k c)", q=128, k=KS)
                nc.sync.dma_start(out=dst, in_=src)
    nc.finalize()
    return nc


def _build_generic(est, closest):
    """Generic program: host-gathered sig^T comes in as an input; the whole
    segment-mean + NN-gather is one dense weight matmul on the PE.
      cov[s, (i,j)] = sum_p wt[p, s] * G[p, (i,j)],  G from sig outer products.
    """
    import concourse.bacc as bacc
    import concourse.mybir as mybir
    from concourse.tile import TileContext

    f32 = mybir.dt.float32
    nc = bacc.Bacc(trn_type="TRN2", target_bir_lowering=False)
    # sig^T per batch: [P_EST, A] split as [KP=8, 128, A]
    sgr = nc.declare_dram_parameter("sgr", [B_LOC, P_EST // 128, 128, A], f32, isOutput=False)
    sgi = nc.declare_dram_parameter("sgi", [B_LOC, P_EST // 128, 128, A], f32, isOutput=False)
    wt = nc.declare_dram_parameter("wt", [P_EST, S], f32, isOutput=False)
    out = nc.declare_dram_parameter("out", [B_LOC, T, S, AA2], f32, isOutput=True)

    KP = P_EST // 128  # contraction chunks
    MS = S // 128      # output subcarrier chunks

    with TileContext(nc) as tc:
        with (
            tc.tile_pool(name="w", bufs=1) as wp,
            tc.tile_pool(name="sig", bufs=2) as sigp,
            tc.tile_pool(name="g", bufs=4) as gp,
            tc.tile_pool(name="ps", bufs=8, space="PSUM") as psp,
            tc.tile_pool(name="f", bufs=2) as fp,
        ):
            w_all = wp.tile([128, KP, S], f32, name="w_all")
            nc.sync.dma_start(
                out=w_all[:], in_=wt[:].rearrange("(k q) s -> q k s", k=KP, q=128)
            )
            for b in range(B_LOC):
                sr = sigp.tile([128, KP, A], f32, tag="sr")
                si = sigp.tile([128, KP, A], f32, tag="si")
                nc.sync.dma_start(
                    out=sr[:], in_=sgr[b].rearrange("k q a -> q k a")
                )
                nc.sync.dma_start(
                    out=si[:], in_=sgi[b].rearrange("k q a -> q k a")
                )

                f = fp.tile([128, MS, A * A, 2], f32, tag="f")
                gtiles = {}
                for k in range(KP):
                    def ii(x):
                        return x[:, k, :, None].to_broadcast([128, A, A])

                    def jj(x):
                        return x[:, k, None, :].to_broadcast([128, A, A])

                    gr = gp.tile([128, A, A], f32, tag=f"gr{k}")
                    gi = gp.tile([128, A, A], f32, tag=f"gi{k}")
                    tt = gp.tile([128, A, A], f32, tag="tt")
                    nc.vector.tensor_mul(gr[:], ii(sr), jj(sr))
                    nc.vector.tensor_mul(tt[:], ii(si), jj(si))
                    nc.vector.tensor_add(gr[:], gr[:], tt[:])
                    nc.vector.tensor_mul(gi[:], ii(si), jj(sr))
                    nc.vector.tensor_mul(tt[:], ii(sr), jj(si))
                    nc.vector.tensor_sub(gi[:], gi[:], tt[:])
                    gtiles[k] = (gr, gi)

                for m in range(MS):
                    for part in range(2):
                        pp = psp.tile([128, A * A], f32, tag="pp")
                        for k in range(KP):
                            g = gtiles[k][part]
                            nc.tensor.matmul(
                                pp[:],
                                lhsT=w_all[:, k, m * 128 : (m + 1) * 128],
                                rhs=g[:].rearrange("q i j -> q (i j)"),
                                start=(k == 0),
                                stop=(k == KP - 1),
                            )
                        nc.vector.tensor_copy(f[:, m, :, part], pp[:])

                src = f[:]
                for t in range(T):
                    dst = out[b, t].rearrange(
                        "(m q) (ij ri) -> q m ij ri", m=MS, q=128, ij=A * A, ri=2
                    )
                    nc.sync.dma_start(out=dst, in_=src)
    nc.finalize()
    return nc


def _get_program(est, closest):
    key = (est.tobytes(), closest.tobytes())
    hit = _cache.get(key)
    if hit is not None:
        return hit
    fast = _fast_path_info(est, closest)
    if fast is not None:
        prog = ("fast", _build_fast(*fast), None)
    else:
        counts = np.zeros(S, dtype=np.float64)
        np.add.at(counts, est[:, 1], 1.0)
        denom = np.maximum(counts, 1.0)
        # wt[p, s] = [sc_p == closest[s]] / denom[closest[s]]
        wt = (
            (est[:, 1][:, None] == closest[None, :]).astype(np.float32)
            / denom[closest][None, :].astype(np.float32)
        )
        prog = ("generic", _build_generic(est, closest), np.ascontiguousarray(wt))
    _cache[key] = prog
    return prog


def kernel(y_real, y_imag, estimation_indices, closest_subcarrier):
    from concourse.bass_utils import run_bass_kernel_spmd

    assert y_real.shape == (B, R, A, T, S), y_real.shape
    est = np.asarray(estimation_indices)
    closest = np.asarray(closest_subcarrier)
    kind, nc, wt = _get_program(est, closest)

    yr = np.ascontiguousarray(np.asarray(y_real, dtype=np.float32)[:, 0])
    yi = np.ascontiguousarray(np.asarray(y_imag, dtype=np.float32)[:, 0])

    if kind == "fast":
        in_maps = [
            {
                "yr": yr[c * B_LOC : (c + 1) * B_LOC],
                "yi": yi[c * B_LOC : (c + 1) * B_LOC],
            }
            for c in range(N_CORES)
        ]
    else:
        sym = est[:, 0].astype(np.int64)
        sc = est[:, 1].astype(np.int64)
        # host gather: sig[b, a, p] = y[b, a, sym_p, sc_p]
        sgr = yr[:, :, sym, sc]  # [B, A, P]
        sgi = yi[:, :, sym, sc]
        # -> [B, KP, 128, A]
        sgr = np.ascontiguousarray(
            sgr.transpose(0, 2, 1).reshape(B, P_EST // 128, 128, A)
        )
        sgi = np.ascontiguousarray(
            sgi.transpose(0, 2, 1).reshape(B, P_EST // 128, 128, A)
        )
        in_maps = [
            {
                "sgr": sgr[c * B_LOC : (c + 1) * B_LOC],
                "sgi": sgi[c * B_LOC : (c + 1) * B_LOC],
                "wt": wt,
            }
            for c in range(N_CORES)
        ]

    res = run_bass_kernel_spmd(nc, in_maps, list(range(N_CORES)))
    parts = [res.results[c]["out"] for c in range(N_CORES)]
    full = np.concatenate(parts, axis=0)  # [B, T, S, AA2]
    return full.view(np.complex64).reshape(B, R, T, S, A, A)



# revision 3
# speedup vs baseline: 5.6220x; 5.6220x over previous
"""Trainium2 Bass kernel for nn_CovarianceEstimator.

Computes, for y [B=16, R=1, A=16, T=14, S=1024] complex (given as separate
real/imag f32 tensors):
  - gather P=1024 pilot positions (sym_p, sc_p) from estimation_indices
  - per-position A x A outer products sig_p sig_p^H
  - unsorted-segment-mean over subcarrier ids sc_p
  - nearest-neighbor expand via closest_subcarrier to all S subcarriers
  - broadcast over T symbols
Output: [B, R, T, S, A, A] complex64.

Sharding: data-parallel over batch; 2 batches per core on 8 cores.

The output tensor is ~470MB but holds only ~17MB of unique data: the T axis
is a pure broadcast and (for the pilot-pattern fast path) subcarrier pairs
share values and the A x A covariance is Hermitian.  The device computes and
writes only the unique data; the host does the (free) broadcast expansion,
NN pair duplication, and Hermitian mirror.

Fast path device program (pilot-pattern indices):
  - positions p = (h, s') for 2 pilot symbols x 512 even subcarriers,
    chunked [k=8][q=128] onto partitions.
  - DVE: 4 tensor_tensor muls per batch compute the Hermitian BAND
    cov[i, (i+d) % 16] for d in 0..8 (144 of 256 entries) using an
    overlapping circulant access pattern on a padded sig tile.
  - PE: identity-stationary matmuls accumulate the h-sum and the
    (aa+bb) / (ba-ab) re/im combinations directly in PSUM.
  - ACT: PSUM -> SBUF evacuation; one 576KB DMA out per batch.

Generic path (any indices): host folds segment-mean + NN-gather into a
dense [P, S] weight matrix applied on the PE (as before), but the device
writes a single [S, AA2] image per batch; host broadcasts over T.
"""

import numpy as np

B, R, A, T, S = 16, 1, 16, 14, 1024
S2 = S // 2           # even (estimated) subcarriers
P_EST = 1024          # number of (sym, sc) estimation positions
N_CORES = 8
B_LOC = B // N_CORES  # 2 batches per core
AA2 = A * A * 2       # interleaved (re, im) row payload per subcarrier
NK = 8                # position chunks of 128 (2 syms x 4 chunks of s')
ND = 9                # Hermitian band width: d = j - i mod A, d in 0..8
APAD = 32             # padded antenna axis (16 data + 9 circular + pad)
NV = A * ND           # 144 band entries per position

_cache = {}


def _fast_path_info(est, closest):
    """Return (sym0, sym1) if indices match the pilot-pattern structure:
    est == meshgrid([sym0, sym1], arange(0, S, 2)) row-major and
    closest == 2*(arange(S)//2).  Else None."""
    if est.shape != (P_EST, 2) or closest.shape != (S,):
        return None
    sc = np.arange(0, S, 2, dtype=est.dtype)
    if not np.array_equal(est[: S // 2, 1], sc):
        return None
    if not np.array_equal(est[S // 2 :, 1], sc):
        return None
    sym0 = int(est[0, 0])
    sym1 = int(est[S // 2, 0])
    if not (0 <= sym0 < T and 0 <= sym1 < T):
        return None
    if not np.all(est[: S // 2, 0] == sym0):
        return None
    if not np.all(est[S // 2 :, 0] == sym1):
        return None
    if not np.array_equal(closest, (2 * (np.arange(S) // 2)).astype(closest.dtype)):
        return None
    return sym0, sym1


def _build_fast():
    """Circulant-band fast path.  Inputs (host-prepared, scaled by sqrt(1/2)):
      spr, spi: [B_LOC, 128, NK, APAD] f32   sig re/im, circularly padded
                over antennas; position p = k*128 + q -> (h, s') = (k//4,
                (k%4)*128 + q).
      ident:    [128, 128] f32 identity (PE stationary).
    Output:
      out: [B_LOC, 4, 128, 2*NV] f32 -- per s'-chunk c, partition q
           (s' = c*128+q): [re band | im band], band = (i, d) i-major.
    """
    import concourse.bacc as bacc
    import concourse.mybir as mybir
    from concourse.tile import TileContext

    f32 = mybir.dt.float32
    nc = bacc.Bacc(trn_type="TRN2", target_bir_lowering=False)
    spr_d = nc.declare_dram_parameter("spr", [B_LOC, 128, NK, APAD], f32, isOutput=False)
    spi_d = nc.declare_dram_parameter("spi", [B_LOC, 128, NK, APAD], f32, isOutput=False)
    id_d = nc.declare_dram_parameter("ident", [128, 128], f32, isOutput=False)
    out = nc.declare_dram_parameter("out", [B_LOC, 4, 128, 2 * NV], f32, isOutput=True)

    with TileContext(nc) as tc:
        with (
            tc.tile_pool(name="const", bufs=1) as cp,
            tc.tile_pool(name="inp", bufs=2) as ip,
            tc.tile_pool(name="g", bufs=2) as gp,
            tc.tile_pool(name="ps", bufs=8, space="PSUM") as pp,
            tc.tile_pool(name="ev", bufs=2) as ep,
        ):
            ident = cp.tile([128, 128], f32, name="ident")
            nc.sync.dma_start(out=ident[:], in_=id_d[:])

            for b in range(B_LOC):
                spr = ip.tile([128, NK, APAD], f32, tag="spr")
                spi = ip.tile([128, NK, APAD], f32, tag="spi")
                nsr = ip.tile([128, NK, A], f32, tag="nsr")
                nc.scalar.dma_start(out=spr[:], in_=spr_d[b])
                nc.gpsimd.dma_start(out=spi[:], in_=spi_d[b])
                # negated re for the -a_i*b_j imag term
                nc.scalar.mul(nsr[:], spr[:, :, 0:A], -1.0)

                # circulant band operand: x[q, k, i+d], strides (i:1, d:1)
                def band(x):
                    base = x[:, :, 0:A, None]
                    apc = type(base)(
                        base.tensor,
                        base.offset,
                        [list(p) for p in x[:].ap[:2]] + [[1, A], [1, ND]],
                    )
                    return apc

                def head(x):  # a_i broadcast over d
                    return x[:, :, 0:A, None].to_broadcast([128, NK, A, ND])

                # G1 = [rr | ir] bands, G2 = [ii | (-r)i] bands
                g1 = gp.tile([128, NK, 2, A, ND], f32, tag="g1")
                g2 = gp.tile([128, NK, 2, A, ND], f32, tag="g2")
                nc.vector.tensor_mul(g1[:, :, 0], head(spr), band(spr))
                nc.vector.tensor_mul(g1[:, :, 1], head(spi), band(spr))
                nc.vector.tensor_mul(g2[:, :, 0], head(spi), band(spi))
                nc.vector.tensor_mul(
                    g2[:, :, 1],
                    nsr[:, :, :, None].to_broadcast([128, NK, A, ND]),
                    band(spi),
                )

                ev = ep.tile([128, 4, 2 * NV], f32, tag="ev")
                for c in range(4):
                    ps = pp.tile([128, 2 * NV], f32, tag="ps")
                    # accumulate h=0/h=1 chunks of [rr|ir] and [ii|-ri]
                    srcs = (g1[:, c], g1[:, 4 + c], g2[:, c], g2[:, 4 + c])
                    for n, s in enumerate(srcs):
                        nc.tensor.matmul(
                            ps[:],
                            lhsT=ident[:],
                            rhs=s.rearrange("q r i d -> q (r i d)"),
                            start=(n == 0),
                            stop=(n == len(srcs) - 1),
                        )
                    nc.scalar.copy(ev[:, c], ps[:])

                nc.sync.dma_start(
                    out=out[b].rearrange("c q v -> q c v"), in_=ev[:]
                )
    nc.finalize()
    return nc


def _build_generic():
    """Generic program: host-gathered sig^T comes in as an input; the whole
    segment-mean + NN-gather is one dense weight matmul on the PE.
      cov[s, (i,j)] = sum_p wt[p, s] * G[p, (i,j)],  G from sig outer products.
    Device writes one [S, AA2] image per batch; host broadcasts over T.
    """
    import concourse.bacc as bacc
    import concourse.mybir as mybir
    from concourse.tile import TileContext

    f32 = mybir.dt.float32
    nc = bacc.Bacc(trn_type="TRN2", target_bir_lowering=False)
    sgr = nc.declare_dram_parameter("sgr", [B_LOC, P_EST // 128, 128, A], f32, isOutput=False)
    sgi = nc.declare_dram_parameter("sgi", [B_LOC, P_EST // 128, 128, A], f32, isOutput=False)
    wt = nc.declare_dram_parameter("wt", [P_EST, S], f32, isOutput=False)
    out = nc.declare_dram_parameter("out", [B_LOC, S, AA2], f32, isOutput=True)

    KP = P_EST // 128  # contraction chunks
    MS = S // 128      # output subcarrier chunks

    with TileContext(nc) as tc:
        with (
            tc.tile_pool(name="w", bufs=1) as wp,
            tc.tile_pool(name="sig", bufs=2) as sigp,
            tc.tile_pool(name="g", bufs=4) as gp,
            tc.tile_pool(name="ps", bufs=8, space="PSUM") as psp,
            tc.tile_pool(name="f", bufs=2) as fp,
        ):
            w_all = wp.tile([128, KP, S], f32, name="w_all")
            nc.sync.dma_start(
                out=w_all[:], in_=wt[:].rearrange("(k q) s -> q k s", k=KP, q=128)
            )
            for b in range(B_LOC):
                sr = sigp.tile([128, KP, A], f32, tag="sr")
                si = sigp.tile([128, KP, A], f32, tag="si")
                nc.sync.dma_start(out=sr[:], in_=sgr[b].rearrange("k q a -> q k a"))
                nc.sync.dma_start(out=si[:], in_=sgi[b].rearrange("k q a -> q k a"))

                f = fp.tile([128, MS, A * A, 2], f32, tag="f")
                gtiles = {}
                for k in range(KP):
                    def ii(x):
                        return x[:, k, :, None].to_broadcast([128, A, A])

                    def jj(x):
                        return x[:, k, None, :].to_broadcast([128, A, A])

                    gr = gp.tile([128, A, A], f32, tag=f"gr{k}")
                    gi = gp.tile([128, A, A], f32, tag=f"gi{k}")
                    tt = gp.tile([128, A, A], f32, tag="tt")
                    nc.vector.tensor_mul(gr[:], ii(sr), jj(sr))
                    nc.vector.tensor_mul(tt[:], ii(si), jj(si))
                    nc.vector.tensor_add(gr[:], gr[:], tt[:])
                    nc.vector.tensor_mul(gi[:], ii(si), jj(sr))
                    nc.vector.tensor_mul(tt[:], ii(sr), jj(si))
                    nc.vector.tensor_sub(gi[:], gi[:], tt[:])
                    gtiles[k] = (gr, gi)

                for m in range(MS):
                    for part in range(2):
                        ppp = psp.tile([128, A * A], f32, tag="pp")
                        for k in range(KP):
                            g = gtiles[k][part]
                            nc.tensor.matmul(
                                ppp[:],
                                lhsT=w_all[:, k, m * 128 : (m + 1) * 128],
                                rhs=g[:].rearrange("q i j -> q (i j)"),
                                start=(k == 0),
                                stop=(k == KP - 1),
                            )
                        nc.vector.tensor_copy(f[:, m, :, part], ppp[:])

                dst = out[b].rearrange(
                    "(m q) (ij ri) -> q m ij ri", m=MS, q=128, ij=A * A, ri=2
                )
                nc.sync.dma_start(out=dst, in_=f[:])
    nc.finalize()
    return nc


def _get_program(est, closest):
    key = (est.tobytes(), closest.tobytes())
    hit = _cache.get(key)
    if hit is not None:
        return hit
    fast = _fast_path_info(est, closest)
    if fast is not None:
        prog = ("fast", _build_fast(), fast)
    else:
        counts = np.zeros(S, dtype=np.float64)
        np.add.at(counts, est[:, 1], 1.0)
        denom = np.maximum(counts, 1.0)
        # wt[p, s] = [sc_p == closest[s]] / denom[closest[s]]
        wtm = (
            (est[:, 1][:, None] == closest[None, :]).astype(np.float32)
            / denom[closest][None, :].astype(np.float32)
        )
        prog = ("generic", _build_generic(), np.ascontiguousarray(wtm))
    _cache[key] = prog
    return prog


def _make_in_maps(kind, extra, yr, yi, est):
    """Build the per-core input maps for the given program kind.
    yr, yi: [B, A, T, S] f32 (R squeezed)."""
    if kind == "fast":
        sym0, sym1 = extra
        scale = np.float32(np.sqrt(0.5))
        # sig[b, h, s', a] = y[b, a, sym_h, 2 s'] * sqrt(1/2)
        def pack(y):
            s = y[:, :, (sym0, sym1), ::2]            # [B, A, 2, S2]
            s = np.transpose(s, (0, 2, 3, 1)) * scale  # [B, 2, S2, A]
            # p = k*128 + q, k = h*4 + c, s' = c*128 + q
            s = s.reshape(B, 2, 4, 128, A).transpose(0, 3, 1, 2, 4)  # [B,128,2,4,A]
            s = s.reshape(B, 128, NK, A)
            sp = np.zeros((B, 128, NK, APAD), dtype=np.float32)
            sp[..., :A] = s
            sp[..., A : A + ND - 1] = s[..., : ND - 1]
            return sp

        spr = pack(yr)
        spi = pack(yi)
        ident = np.eye(128, dtype=np.float32)
        return [
            {
                "spr": spr[c * B_LOC : (c + 1) * B_LOC],
                "spi": spi[c * B_LOC : (c + 1) * B_LOC],
                "ident": ident,
            }
            for c in range(N_CORES)
        ]
    else:
        wtm = extra
        sym = est[:, 0].astype(np.int64)
        sc = est[:, 1].astype(np.int64)
        sgr = yr[:, :, sym, sc]  # [B, A, P]
        sgi = yi[:, :, sym, sc]
        sgr = np.ascontiguousarray(
            sgr.transpose(0, 2, 1).reshape(B, P_EST // 128, 128, A)
        )
        sgi = np.ascontiguousarray(
            sgi.transpose(0, 2, 1).reshape(B, P_EST // 128, 128, A)
        )
        return [
            {
                "sgr": sgr[c * B_LOC : (c + 1) * B_LOC],
                "sgi": sgi[c * B_LOC : (c + 1) * B_LOC],
                "wt": wtm,
            }
            for c in range(N_CORES)
        ]


_II, _DD = np.meshgrid(np.arange(A), np.arange(ND), indexing="ij")
_JJ = (_II + _DD) % A


def kernel(y_real, y_imag, estimation_indices, closest_subcarrier):
    from concourse.bass_utils import run_bass_kernel_spmd

    assert y_real.shape == (B, R, A, T, S), y_real.shape
    est = np.asarray(estimation_indices)
    closest = np.asarray(closest_subcarrier)
    kind, nc, extra = _get_program(est, closest)

    yr = np.ascontiguousarray(np.asarray(y_real, dtype=np.float32)[:, 0])
    yi = np.ascontiguousarray(np.asarray(y_imag, dtype=np.float32)[:, 0])
    in_maps = _make_in_maps(kind, extra, yr, yi, est)

    res = run_bass_kernel_spmd(nc, in_maps, list(range(N_CORES)))
    parts = [res.results[c]["out"] for c in range(N_CORES)]
    full = np.concatenate(parts, axis=0)

    if kind == "fast":
        # full: [B, 4, 128, 2*NV] -> band values v[b, s', i, d]
        full = full.reshape(B, S2, 2, A, ND)
        v = (full[:, :, 0] + 1j * full[:, :, 1]).astype(np.complex64)
        cov_half = np.empty((B, S2, A, A), dtype=np.complex64)
        cov_half[:, :, _II, _JJ] = v
        cov_half[:, :, _JJ, _II] = np.conj(v)
        cov = np.repeat(cov_half, 2, axis=1)  # NN expand to all S
    else:
        # full: [B, S, AA2] interleaved (ij, ri)
        cov = full.view(np.complex64).reshape(B, S, A, A)

    out = np.broadcast_to(
        cov.reshape(B, 1, 1, S, A, A), (B, R, T, S, A, A)
    )
    return np.ascontiguousarray(out)


# revision 6
# speedup vs baseline: 8.5394x; 1.5189x over previous
"""Trainium2 Bass kernel for nn_CovarianceEstimator.

Computes, for y [B=16, R=1, A=16, T=14, S=1024] complex (given as separate
real/imag f32 tensors):
  - gather P=1024 pilot positions (sym_p, sc_p) from estimation_indices
  - per-position A x A outer products sig_p sig_p^H
  - unsorted-segment-mean over subcarrier ids sc_p
  - nearest-neighbor expand via closest_subcarrier to all S subcarriers
  - broadcast over T symbols
Output: [B, R, T, S, A, A] complex64.

Sharding: data-parallel over batch; 2 batches per core on 8 cores.

The output tensor is ~470MB but holds only ~17MB of unique data: the T axis
is a pure broadcast and (for the pilot-pattern fast path) subcarrier pairs
share values and the A x A covariance is Hermitian.  The device computes and
writes only the unique data; the host does the (free) broadcast expansion,
NN pair duplication, and Hermitian mirror.

Fast path device program (pilot-pattern indices):
  - positions p = (h, s') for 2 pilot symbols x 512 even subcarriers,
    chunked [k=8][q=128] onto partitions.
  - DVE: 4 tensor_tensor muls per batch compute the Hermitian BAND
    cov[i, (i+d) % 16] for d in 0..8 (144 of 256 entries) using an
    overlapping circulant access pattern on a padded sig tile.
  - PE: identity-stationary matmuls accumulate the h-sum and the
    (aa+bb) / (ba-ab) re/im combinations directly in PSUM.
  - ACT: PSUM -> SBUF evacuation; one 576KB DMA out per batch.

Generic path (any indices): host folds segment-mean + NN-gather into a
dense [P, S] weight matrix applied on the PE (as before), but the device
writes a single [S, AA2] image per batch; host broadcasts over T.
"""

import numpy as np

B, R, A, T, S = 16, 1, 16, 14, 1024
S2 = S // 2           # even (estimated) subcarriers
P_EST = 1024          # number of (sym, sc) estimation positions
N_CORES = 8
B_LOC = B // N_CORES  # 2 batches per core
AA2 = A * A * 2       # interleaved (re, im) row payload per subcarrier
NK = 8                # position chunks of 128 (2 syms x 4 chunks of s')
ND = 9                # Hermitian band width: d = j - i mod A, d in 0..8
APAD = 32             # padded antenna axis (16 data + 9 circular + pad)
NV = A * ND           # 144 band entries per position

_cache = {}


def _fast_path_info(est, closest):
    """Return (sym0, sym1) if indices match the pilot-pattern structure:
    est == meshgrid([sym0, sym1], arange(0, S, 2)) row-major and
    closest == 2*(arange(S)//2).  Else None."""
    if est.shape != (P_EST, 2) or closest.shape != (S,):
        return None
    sc = np.arange(0, S, 2, dtype=est.dtype)
    if not np.array_equal(est[: S // 2, 1], sc):
        return None
    if not np.array_equal(est[S // 2 :, 1], sc):
        return None
    sym0 = int(est[0, 0])
    sym1 = int(est[S // 2, 0])
    if not (0 <= sym0 < T and 0 <= sym1 < T):
        return None
    if not np.all(est[: S // 2, 0] == sym0):
        return None
    if not np.all(est[S // 2 :, 0] == sym1):
        return None
    if not np.array_equal(closest, (2 * (np.arange(S) // 2)).astype(closest.dtype)):
        return None
    return sym0, sym1


def _build_fast():
    """Circulant-band fast path.  Inputs (host-prepared fp16, scaled by
    sqrt(1/2)); position p = k*128 + q -> (h, s') = (k//4, (k%4)*128 + q):
      spr, spi: [B_LOC, 128, NK, APAD]  sig re/im, circularly padded over
                antennas (24 used).
      ssr, ssi: same, shifted by one antenna (ss[i] = sp[i+1]) so odd-d
                band reads stay 4B-aligned (DVE 2x perf mode).
      nsr:      [B_LOC, 128, NK, A]  negated re (for the -a_i*b_j term).
      ident:    [128, 128] fp16 identity (PE stationary).
    Output:
      out: [B_LOC, 4, 128, 2*NV] f32 -- per s'-chunk c, partition q
           (s' = c*128+q): [re band | im band], band = (d, i) d-major.
    """
    import concourse.bacc as bacc
    import concourse.mybir as mybir
    from concourse.tile import TileContext

    f32 = mybir.dt.float32
    f16 = mybir.dt.float16
    nc = bacc.Bacc(trn_type="TRN2", target_bir_lowering=False)
    spr_d = nc.declare_dram_parameter("spr", [B_LOC, 128, NK, APAD], f16, isOutput=False)
    spi_d = nc.declare_dram_parameter("spi", [B_LOC, 128, NK, APAD], f16, isOutput=False)
    ssr_d = nc.declare_dram_parameter("ssr", [B_LOC, 128, NK, APAD], f16, isOutput=False)
    ssi_d = nc.declare_dram_parameter("ssi", [B_LOC, 128, NK, APAD], f16, isOutput=False)
    nsr_d = nc.declare_dram_parameter("nsr", [B_LOC, 128, NK, A], f16, isOutput=False)
    id_d = nc.declare_dram_parameter("ident", [128, 128], f16, isOutput=False)
    out = nc.declare_dram_parameter("out", [B_LOC, 4, 128, 2 * NV], f32, isOutput=True)

    NDE = (ND + 1) // 2  # even d values: 0,2,4,6,8
    NDO = ND // 2        # odd d values: 1,3,5,7

    with TileContext(nc) as tc:
        with (
            tc.tile_pool(name="const", bufs=1) as cp,
            tc.tile_pool(name="inp", bufs=2) as ip,
            tc.tile_pool(name="g", bufs=2) as gp,
            tc.tile_pool(name="ps", bufs=8, space="PSUM") as pp,
            tc.tile_pool(name="ev", bufs=2) as ep,
        ):
            ident = cp.tile([128, 128], f16, name="ident")
            nc.sync.dma_start(out=ident[:], in_=id_d[:])

            for b in range(B_LOC):
                spr = ip.tile([128, NK, APAD], f16, tag="spr")
                spi = ip.tile([128, NK, APAD], f16, tag="spi")
                ssr = ip.tile([128, NK, APAD], f16, tag="ssr")
                ssi = ip.tile([128, NK, APAD], f16, tag="ssi")
                nsr = ip.tile([128, NK, A], f16, tag="nsr")
                nc.scalar.dma_start(out=spr[:], in_=spr_d[b])
                nc.scalar.dma_start(out=ssr[:], in_=ssr_d[b])
                nc.gpsimd.dma_start(out=spi[:], in_=spi_d[b])
                nc.gpsimd.dma_start(out=ssi[:], in_=ssi_d[b])
                nc.gpsimd.dma_start(out=nsr[:], in_=nsr_d[b])

                # band operands x[q, k, d, i] = base[q, k, i + d]:
                #   even d from sp (offset d), odd d from ss (offset d-1);
                # both give innermost unit-stride, 4B-aligned runs of 16.
                def band_e(x):  # d in {0,2,4,6,8}
                    base = x[:]
                    return type(base)(
                        base.tensor,
                        base.offset,
                        [list(p) for p in base.ap[:2]] + [[2, NDE], [1, A]],
                    )

                def band_o(x):  # d in {1,3,5,7}, x pre-shifted by 1
                    base = x[:]
                    return type(base)(
                        base.tensor,
                        base.offset,
                        [list(p) for p in base.ap[:2]] + [[2, NDO], [1, A]],
                    )

                def head(x, nd):  # a_i broadcast over d (outer dim)
                    return x[:, :, None, 0:A].to_broadcast([128, NK, nd, A])

                # G = [slot 0: rr+? | slot 1: ir/-ri] bands, (d, i) d-major
                g1 = gp.tile([128, NK, 2, ND, A], f16, tag="g1")
                g2 = gp.tile([128, NK, 2, ND, A], f16, tag="g2")
                for g, hd, tl in (
                    (g1[:, :, 0], spr, (spr, ssr)),   # a_i a_j
                    (g1[:, :, 1], spi, (spr, ssr)),   # b_i a_j
                    (g2[:, :, 0], spi, (spi, ssi)),   # b_i b_j
                    (g2[:, :, 1], nsr, (spi, ssi)),   # -a_i b_j
                ):
                    nc.vector.tensor_mul(g[:, :, 0:ND:2], head(hd, NDE), band_e(tl[0]))
                    nc.vector.tensor_mul(g[:, :, 1:ND:2], head(hd, NDO), band_o(tl[1]))

                ev = ep.tile([128, 4, 2 * NV], f32, tag="ev")
                for c in range(4):
                    ps = pp.tile([128, 2 * NV], f32, tag="ps")
                    # accumulate h=0/h=1 chunks of [rr|ir] and [ii|-ri]
                    srcs = (g1[:, c], g1[:, 4 + c], g2[:, c], g2[:, 4 + c])
                    for n, s in enumerate(srcs):
                        nc.tensor.matmul(
                            ps[:],
                            lhsT=ident[:],
                            rhs=s.rearrange("q r d i -> q (r d i)"),
                            start=(n == 0),
                            stop=(n == len(srcs) - 1),
                        )
                    nc.scalar.copy(ev[:, c], ps[:])

                nc.sync.dma_start(
                    out=out[b].rearrange("c q v -> q c v"), in_=ev[:]
                )
    nc.finalize()
    return nc


def _build_generic():
    """Generic program: host-gathered sig^T comes in as an input; the whole
    segment-mean + NN-gather is one dense weight matmul on the PE.
      cov[s, (i,j)] = sum_p wt[p, s] * G[p, (i,j)],  G from sig outer products.
    Device writes one [S, AA2] image per batch; host broadcasts over T.
    """
    import concourse.bacc as bacc
    import concourse.mybir as mybir
    from concourse.tile import TileContext

    f32 = mybir.dt.float32
    nc = bacc.Bacc(trn_type="TRN2", target_bir_lowering=False)
    sgr = nc.declare_dram_parameter("sgr", [B_LOC, P_EST // 128, 128, A], f32, isOutput=False)
    sgi = nc.declare_dram_parameter("sgi", [B_LOC, P_EST // 128, 128, A], f32, isOutput=False)
    wt = nc.declare_dram_parameter("wt", [P_EST, S], f32, isOutput=False)
    out = nc.declare_dram_parameter("out", [B_LOC, S, AA2], f32, isOutput=True)

    KP = P_EST // 128  # contraction chunks
    MS = S // 128      # output subcarrier chunks

    with TileContext(nc) as tc:
        with (
            tc.tile_pool(name="w", bufs=1) as wp,
            tc.tile_pool(name="sig", bufs=2) as sigp,
            tc.tile_pool(name="g", bufs=4) as gp,
            tc.tile_pool(name="ps", bufs=8, space="PSUM") as psp,
            tc.tile_pool(name="f", bufs=2) as fp,
        ):
            w_all = wp.tile([128, KP, S], f32, name="w_all")
            nc.sync.dma_start(
                out=w_all[:], in_=wt[:].rearrange("(k q) s -> q k s", k=KP, q=128)
            )
            for b in range(B_LOC):
                sr = sigp.tile([128, KP, A], f32, tag="sr")
                si = sigp.tile([128, KP, A], f32, tag="si")
                nc.sync.dma_start(out=sr[:], in_=sgr[b].rearrange("k q a -> q k a"))
                nc.sync.dma_start(out=si[:], in_=sgi[b].rearrange("k q a -> q k a"))

                f = fp.tile([128, MS, A * A, 2], f32, tag="f")
                gtiles = {}
                for k in range(KP):
                    def ii(x):
                        return x[:, k, :, None].to_broadcast([128, A, A])

                    def jj(x):
                        return x[:, k, None, :].to_broadcast([128, A, A])

                    gr = gp.tile([128, A, A], f32, tag=f"gr{k}")
                    gi = gp.tile([128, A, A], f32, tag=f"gi{k}")
                    tt = gp.tile([128, A, A], f32, tag="tt")
                    nc.vector.tensor_mul(gr[:], ii(sr), jj(sr))
                    nc.vector.tensor_mul(tt[:], ii(si), jj(si))
                    nc.vector.tensor_add(gr[:], gr[:], tt[:])
                    nc.vector.tensor_mul(gi[:], ii(si), jj(sr))
                    nc.vector.tensor_mul(tt[:], ii(sr), jj(si))
                    nc.vector.tensor_sub(gi[:], gi[:], tt[:])
                    gtiles[k] = (gr, gi)

                for m in range(MS):
                    for part in range(2):
                        ppp = psp.tile([128, A * A], f32, tag="pp")
                        for k in range(KP):
                            g = gtiles[k][part]
                            nc.tensor.matmul(
                                ppp[:],
                                lhsT=w_all[:, k, m * 128 : (m + 1) * 128],
                                rhs=g[:].rearrange("q i j -> q (i j)"),
                                start=(k == 0),
                                stop=(k == KP - 1),
                            )
                        nc.vector.tensor_copy(f[:, m, :, part], ppp[:])

                dst = out[b].rearrange(
                    "(m q) (ij ri) -> q m ij ri", m=MS, q=128, ij=A * A, ri=2
                )
                nc.sync.dma_start(out=dst, in_=f[:])
    nc.finalize()
    return nc


def _get_program(est, closest):
    key = (est.tobytes(), closest.tobytes())
    hit = _cache.get(key)
    if hit is not None:
        return hit
    fast = _fast_path_info(est, closest)
    if fast is not None:
        prog = ("fast", _build_fast(), fast)
    else:
        counts = np.zeros(S, dtype=np.float64)
        np.add.at(counts, est[:, 1], 1.0)
        denom = np.maximum(counts, 1.0)
        # wt[p, s] = [sc_p == closest[s]] / denom[closest[s]]
        wtm = (
            (est[:, 1][:, None] == closest[None, :]).astype(np.float32)
            / denom[closest][None, :].astype(np.float32)
        )
        prog = ("generic", _build_generic(), np.ascontiguousarray(wtm))
    _cache[key] = prog
    return prog


def _make_in_maps(kind, extra, yr, yi, est):
    """Build the per-core input maps for the given program kind.
    yr, yi: [B, A, T, S] f32 (R squeezed)."""
    if kind == "fast":
        sym0, sym1 = extra
        scale = np.float32(np.sqrt(0.5))
        # sig[b, h, s', a] = y[b, a, sym_h, 2 s'] * sqrt(1/2)
        def pack(y):
            s = y[:, :, (sym0, sym1), ::2]            # [B, A, 2, S2]
            s = np.transpose(s, (0, 2, 3, 1)) * scale  # [B, 2, S2, A]
            # p = k*128 + q, k = h*4 + c, s' = c*128 + q
            s = s.reshape(B, 2, 4, 128, A).transpose(0, 3, 1, 2, 4)  # [B,128,2,4,A]
            s = s.reshape(B, 128, NK, A)
            sp = np.zeros((B, 128, NK, APAD), dtype=np.float16)
            sp[..., :A] = s
            sp[..., A : A + ND - 1] = s[..., : ND - 1]
            ss = np.zeros_like(sp)
            ss[..., : A + ND - 2] = sp[..., 1 : A + ND - 1]
            return sp, ss

        spr, ssr = pack(yr)
        spi, ssi = pack(yi)
        nsr = np.ascontiguousarray(-spr[..., :A])
        ident = np.eye(128, dtype=np.float16)
        return [
            {
                "spr": spr[c * B_LOC : (c + 1) * B_LOC],
                "spi": spi[c * B_LOC : (c + 1) * B_LOC],
                "ssr": ssr[c * B_LOC : (c + 1) * B_LOC],
                "ssi": ssi[c * B_LOC : (c + 1) * B_LOC],
                "nsr": nsr[c * B_LOC : (c + 1) * B_LOC],
                "ident": ident,
            }
            for c in range(N_CORES)
        ]
    else:
        wtm = extra
        sym = est[:, 0].astype(np.int64)
        sc = est[:, 1].astype(np.int64)
        sgr = yr[:, :, sym, sc]  # [B, A, P]
        sgi = yi[:, :, sym, sc]
        sgr = np.ascontiguousarray(
            sgr.transpose(0, 2, 1).reshape(B, P_EST // 128, 128, A)
        )
        sgi = np.ascontiguousarray(
            sgi.transpose(0, 2, 1).reshape(B, P_EST // 128, 128, A)
        )
        return [
            {
                "sgr": sgr[c * B_LOC : (c + 1) * B_LOC],
                "sgi": sgi[c * B_LOC : (c + 1) * B_LOC],
                "wt": wtm,
            }
            for c in range(N_CORES)
        ]


_DD, _II = np.meshgrid(np.arange(ND), np.arange(A), indexing="ij")
_JJ = (_II + _DD) % A


def kernel(y_real, y_imag, estimation_indices, closest_subcarrier):
    from concourse.bass_utils import run_bass_kernel_spmd

    assert y_real.shape == (B, R, A, T, S), y_real.shape
    est = np.asarray(estimation_indices)
    closest = np.asarray(closest_subcarrier)
    kind, nc, extra = _get_program(est, closest)

    yr = np.ascontiguousarray(np.asarray(y_real, dtype=np.float32)[:, 0])
    yi = np.ascontiguousarray(np.asarray(y_imag, dtype=np.float32)[:, 0])
    in_maps = _make_in_maps(kind, extra, yr, yi, est)

    res = run_bass_kernel_spmd(nc, in_maps, list(range(N_CORES)))
    parts = [res.results[c]["out"] for c in range(N_CORES)]
    full = np.concatenate(parts, axis=0)

    if kind == "fast":
        # full: [B, 4, 128, 2*NV] -> band values v[b, s', d, i]
        full = full.reshape(B, S2, 2, ND, A)
        v = (full[:, :, 0] + 1j * full[:, :, 1]).astype(np.complex64)
        cov_half = np.empty((B, S2, A, A), dtype=np.complex64)
        cov_half[:, :, _II, _JJ] = v
        cov_half[:, :, _JJ, _II] = np.conj(v)
        cov = np.repeat(cov_half, 2, axis=1)  # NN expand to all S
    else:
        # full: [B, S, AA2] interleaved (ij, ri)
        cov = full.view(np.complex64).reshape(B, S, A, A)

    out = np.broadcast_to(
        cov.reshape(B, 1, 1, S, A, A), (B, R, T, S, A, A)
    )
    return np.ascontiguousarray(out)


# revision 11
# speedup vs baseline: 8.7637x; 1.0263x over previous
"""Trainium2 Bass kernel for nn_CovarianceEstimator.

Computes, for y [B=16, R=1, A=16, T=14, S=1024] complex (given as separate
real/imag f32 tensors):
  - gather P=1024 pilot positions (sym_p, sc_p) from estimation_indices
  - per-position A x A outer products sig_p sig_p^H
  - unsorted-segment-mean over subcarrier ids sc_p
  - nearest-neighbor expand via closest_subcarrier to all S subcarriers
  - broadcast over T symbols
Output: [B, R, T, S, A, A] complex64.

Sharding: data-parallel over batch; 2 batches per core on 8 cores.

The output tensor is ~470MB but holds only ~17MB of unique data: the T axis
is a pure broadcast and (for the pilot-pattern fast path) subcarrier pairs
share values and the A x A covariance is Hermitian.  The device computes and
writes only the unique data; the host does the (free) broadcast expansion,
NN pair duplication, and Hermitian mirror.

Fast path device program (pilot-pattern indices):
  - positions p = (h, s') for 2 pilot symbols x 512 even subcarriers,
    chunked [k=8][q=128] onto partitions.
  - DVE: 4 tensor_tensor muls per batch compute the Hermitian BAND
    cov[i, (i+d) % 16] for d in 0..8 (144 of 256 entries) using an
    overlapping circulant access pattern on a padded sig tile.
  - PE: identity-stationary matmuls accumulate the h-sum and the
    (aa+bb) / (ba-ab) re/im combinations directly in PSUM.
  - ACT: PSUM -> SBUF evacuation; one 576KB DMA out per batch.

Generic path (any indices): host folds segment-mean + NN-gather into a
dense [P, S] weight matrix applied on the PE (as before), but the device
writes a single [S, AA2] image per batch; host broadcasts over T.
"""

import numpy as np

B, R, A, T, S = 16, 1, 16, 14, 1024
S2 = S // 2           # even (estimated) subcarriers
P_EST = 1024          # number of (sym, sc) estimation positions
N_CORES = 8
B_LOC = B // N_CORES  # 2 batches per core
AA2 = A * A * 2       # interleaved (re, im) row payload per subcarrier
NK = 8                # position chunks of 128 (2 syms x 4 chunks of s')
ND = 9                # Hermitian band width: d = j - i mod A, d in 0..8
APAD = 32             # padded antenna axis (16 data + 9 circular + pad)
NV = A * ND           # 144 band entries per position

_cache = {}


def _fast_path_info(est, closest):
    """Return (sym0, sym1) if indices match the pilot-pattern structure:
    est == meshgrid([sym0, sym1], arange(0, S, 2)) row-major and
    closest == 2*(arange(S)//2).  Else None."""
    if est.shape != (P_EST, 2) or closest.shape != (S,):
        return None
    sc = np.arange(0, S, 2, dtype=est.dtype)
    if not np.array_equal(est[: S // 2, 1], sc):
        return None
    if not np.array_equal(est[S // 2 :, 1], sc):
        return None
    sym0 = int(est[0, 0])
    sym1 = int(est[S // 2, 0])
    if not (0 <= sym0 < T and 0 <= sym1 < T):
        return None
    if not np.all(est[: S // 2, 0] == sym0):
        return None
    if not np.all(est[S // 2 :, 0] == sym1):
        return None
    if not np.array_equal(closest, (2 * (np.arange(S) // 2)).astype(closest.dtype)):
        return None
    return sym0, sym1


def _build_fast():
    """Circulant-band fast path.  Inputs (host-prepared fp16, scaled by
    sqrt(1/2)); position p = k*128 + q -> (h, s') = (k//4, (k%4)*128 + q):
      spr, spi: [B_LOC, 128, NK, APAD]  sig re/im, circularly padded over
                antennas (24 used).
      ssr, ssi: same, shifted by one antenna (ss[i] = sp[i+1]) so odd-d
                band reads stay 4B-aligned (DVE 2x perf mode).
      nsr:      [B_LOC, 128, NK, A]  negated re (for the -a_i*b_j term).
      ident:    [128, 128] fp16 identity (PE stationary).
    Output:
      out: [B_LOC, 4, 128, 2*NV] f32 -- per s'-chunk c, partition q
           (s' = c*128+q): [re band | im band], band = (d, i) d-major.
    """
    import concourse.bacc as bacc
    import concourse.mybir as mybir
    from concourse.tile import TileContext

    f32 = mybir.dt.float32
    f16 = mybir.dt.float16
    nc = bacc.Bacc(trn_type="TRN2", target_bir_lowering=False)
    # packed input image: segs [spr, ssr, spi, ssi, nsr(padded)]
    in_d = nc.declare_dram_parameter("inall", [B_LOC, 128, 5, NK, APAD], f16, isOutput=False)
    id_d = nc.declare_dram_parameter("ident", [128, 128], f16, isOutput=False)
    out = nc.declare_dram_parameter("out", [B_LOC, 4, 128, 2 * NV], f16, isOutput=True)

    NDE = (ND + 1) // 2  # even d values: 0,2,4,6,8
    NDO = ND // 2        # odd d values: 1,3,5,7

    with TileContext(nc) as tc:
        with (
            tc.tile_pool(name="const", bufs=1) as cp,
            tc.tile_pool(name="inp", bufs=1) as ip,
            tc.tile_pool(name="g", bufs=2) as gp,
            tc.tile_pool(name="ps", bufs=8, space="PSUM") as pp,
            tc.tile_pool(name="ev", bufs=2) as ep,
        ):
            ident = cp.tile([128, 128], f16, name="ident")
            nc.sync.dma_start(out=ident[:], in_=id_d[:])
            inall = ip.tile([128, B_LOC, 5, NK, APAD], f16, name="inall")
            nc.scalar.dma_start(
                out=inall[:, 0], in_=in_d[0]
            )
            nc.gpsimd.dma_start(
                out=inall[:, 1], in_=in_d[1]
            )

            for b in range(B_LOC):
                spr = inall[:, b, 0]
                ssr = inall[:, b, 1]
                spi = inall[:, b, 2]
                ssi = inall[:, b, 3]
                nsr = inall[:, b, 4]

                # band operands x[q, k, d, i] = base[q, k, i + d]:
                #   even d from sp (offset d), odd d from ss (offset d-1);
                # both give innermost unit-stride, 4B-aligned runs of 16.
                def band_e(base):  # d in {0,2,4,6,8}
                    return type(base)(
                        base.tensor,
                        base.offset,
                        [list(p) for p in base.ap[:2]] + [[2, NDE], [1, A]],
                    )

                def band_o(base):  # d in {1,3,5,7}, base pre-shifted by 1
                    return type(base)(
                        base.tensor,
                        base.offset,
                        [list(p) for p in base.ap[:2]] + [[2, NDO], [1, A]],
                    )

                def head(x, nd):  # a_i broadcast over d (outer dim)
                    return x[:, :, 0:A].unsqueeze(2).to_broadcast([128, NK, nd, A])

                # G = [slot 0: rr+? | slot 1: ir/-ri] bands, (d, i) d-major
                g1 = gp.tile([128, NK, 2, ND, A], f16, tag="g1")
                g2 = gp.tile([128, NK, 2, ND, A], f16, tag="g2")
                for g, hd, tl in (
                    (g1[:, :, 0], spr, (spr, ssr)),   # a_i a_j
                    (g1[:, :, 1], spi, (spr, ssr)),   # b_i a_j
                    (g2[:, :, 0], spi, (spi, ssi)),   # b_i b_j
                    (g2[:, :, 1], nsr, (spi, ssi)),   # -a_i b_j
                ):
                    nc.vector.tensor_mul(g[:, :, 0:ND:2], head(hd, NDE), band_e(tl[0]))
                    nc.vector.tensor_mul(g[:, :, 1:ND:2], head(hd, NDO), band_o(tl[1]))

                ev = ep.tile([128, 4, 2 * NV], f16, tag="ev")
                for c in range(4):
                    ps = pp.tile([128, 2 * NV], f32, tag="ps")
                    # accumulate h=0/h=1 chunks of [rr|ir] and [ii|-ri]
                    srcs = (g1[:, c], g1[:, 4 + c], g2[:, c], g2[:, 4 + c])
                    for n, s in enumerate(srcs):
                        nc.tensor.matmul(
                            ps[:],
                            lhsT=ident[:],
                            rhs=s.rearrange("q r d i -> q (r d i)"),
                            start=(n == 0),
                            stop=(n == len(srcs) - 1),
                        )
                    nc.scalar.copy(ev[:, c], ps[:])
                    if c % 2 == 1:  # ship each completed half immediately
                        nc.sync.dma_start(
                            out=out[b, c - 1 : c + 1].rearrange("c q v -> q c v"),
                            in_=ev[:, c - 1 : c + 1],
                        )
    nc.finalize()
    return nc


def _build_generic():
    """Generic program: host-gathered sig^T comes in as an input; the whole
    segment-mean + NN-gather is one dense weight matmul on the PE.
      cov[s, (i,j)] = sum_p wt[p, s] * G[p, (i,j)],  G from sig outer products.
    Device writes one [S, AA2] image per batch; host broadcasts over T.
    """
    import concourse.bacc as bacc
    import concourse.mybir as mybir
    from concourse.tile import TileContext

    f32 = mybir.dt.float32
    nc = bacc.Bacc(trn_type="TRN2", target_bir_lowering=False)
    sgr = nc.declare_dram_parameter("sgr", [B_LOC, P_EST // 128, 128, A], f32, isOutput=False)
    sgi = nc.declare_dram_parameter("sgi", [B_LOC, P_EST // 128, 128, A], f32, isOutput=False)
    wt = nc.declare_dram_parameter("wt", [P_EST, S], f32, isOutput=False)
    out = nc.declare_dram_parameter("out", [B_LOC, S, AA2], f32, isOutput=True)

    KP = P_EST // 128  # contraction chunks
    MS = S // 128      # output subcarrier chunks

    with TileContext(nc) as tc:
        with (
            tc.tile_pool(name="w", bufs=1) as wp,
            tc.tile_pool(name="sig", bufs=2) as sigp,
            tc.tile_pool(name="g", bufs=4) as gp,
            tc.tile_pool(name="ps", bufs=8, space="PSUM") as psp,
            tc.tile_pool(name="f", bufs=2) as fp,
        ):
            w_all = wp.tile([128, KP, S], f32, name="w_all")
            nc.sync.dma_start(
                out=w_all[:], in_=wt[:].rearrange("(k q) s -> q k s", k=KP, q=128)
            )
            for b in range(B_LOC):
                sr = sigp.tile([128, KP, A], f32, tag="sr")
                si = sigp.tile([128, KP, A], f32, tag="si")
                nc.sync.dma_start(out=sr[:], in_=sgr[b].rearrange("k q a -> q k a"))
                nc.sync.dma_start(out=si[:], in_=sgi[b].rearrange("k q a -> q k a"))

                f = fp.tile([128, MS, A * A, 2], f32, tag="f")
                gtiles = {}
                for k in range(KP):
                    def ii(x):
                        return x[:, k, :, None].to_broadcast([128, A, A])

                    def jj(x):
                        return x[:, k, None, :].to_broadcast([128, A, A])

                    gr = gp.tile([128, A, A], f32, tag=f"gr{k}")
                    gi = gp.tile([128, A, A], f32, tag=f"gi{k}")
                    tt = gp.tile([128, A, A], f32, tag="tt")
                    nc.vector.tensor_mul(gr[:], ii(sr), jj(sr))
                    nc.vector.tensor_mul(tt[:], ii(si), jj(si))
                    nc.vector.tensor_add(gr[:], gr[:], tt[:])
                    nc.vector.tensor_mul(gi[:], ii(si), jj(sr))
                    nc.vector.tensor_mul(tt[:], ii(sr), jj(si))
                    nc.vector.tensor_sub(gi[:], gi[:], tt[:])
                    gtiles[k] = (gr, gi)

                for m in range(MS):
                    for part in range(2):
                        ppp = psp.tile([128, A * A], f32, tag="pp")
                        for k in range(KP):
                            g = gtiles[k][part]
                            nc.tensor.matmul(
                                ppp[:],
                                lhsT=w_all[:, k, m * 128 : (m + 1) * 128],
                                rhs=g[:].rearrange("q i j -> q (i j)"),
                                start=(k == 0),
                                stop=(k == KP - 1),
                            )
                        nc.vector.tensor_copy(f[:, m, :, part], ppp[:])

                dst = out[b].rearrange(
                    "(m q) (ij ri) -> q m ij ri", m=MS, q=128, ij=A * A, ri=2
                )
                nc.sync.dma_start(out=dst, in_=f[:])
    nc.finalize()
    return nc


def _get_program(est, closest):
    key = (est.tobytes(), closest.tobytes())
    hit = _cache.get(key)
    if hit is not None:
        return hit
    fast = _fast_path_info(est, closest)
    if fast is not None:
        prog = ("fast", _build_fast(), fast)
    else:
        counts = np.zeros(S, dtype=np.float64)
        np.add.at(counts, est[:, 1], 1.0)
        denom = np.maximum(counts, 1.0)
        # wt[p, s] = [sc_p == closest[s]] / denom[closest[s]]
        wtm = (
            (est[:, 1][:, None] == closest[None, :]).astype(np.float32)
            / denom[closest][None, :].astype(np.float32)
        )
        prog = ("generic", _build_generic(), np.ascontiguousarray(wtm))
    _cache[key] = prog
    return prog


def _make_in_maps(kind, extra, yr, yi, est):
    """Build the per-core input maps for the given program kind.
    yr, yi: [B, A, T, S] f32 (R squeezed)."""
    if kind == "fast":
        sym0, sym1 = extra
        scale = np.float32(np.sqrt(0.5))
        # sig[b, h, s', a] = y[b, a, sym_h, 2 s'] * sqrt(1/2)
        def pack(y):
            s = y[:, :, (sym0, sym1), ::2]            # [B, A, 2, S2]
            s = np.transpose(s, (0, 2, 3, 1)) * scale  # [B, 2, S2, A]
            # p = k*128 + q, k = h*4 + c, s' = c*128 + q
            s = s.reshape(B, 2, 4, 128, A).transpose(0, 3, 1, 2, 4)  # [B,128,2,4,A]
            s = s.reshape(B, 128, NK, A)
            sp = np.zeros((B, 128, NK, APAD), dtype=np.float16)
            sp[..., :A] = s
            sp[..., A : A + ND - 1] = s[..., : ND - 1]
            ss = np.zeros_like(sp)
            ss[..., : A + ND - 2] = sp[..., 1 : A + ND - 1]
            return sp, ss

        spr, ssr = pack(yr)
        spi, ssi = pack(yi)
        inall = np.zeros((B, 128, 5, NK, APAD), dtype=np.float16)
        inall[:, :, 0] = spr
        inall[:, :, 1] = ssr
        inall[:, :, 2] = spi
        inall[:, :, 3] = ssi
        inall[:, :, 4, :, :A] = -spr[..., :A]
        ident = np.eye(128, dtype=np.float16)
        return [
            {
                "inall": inall[c * B_LOC : (c + 1) * B_LOC],
                "ident": ident,
            }
            for c in range(N_CORES)
        ]
    else:
        wtm = extra
        sym = est[:, 0].astype(np.int64)
        sc = est[:, 1].astype(np.int64)
        sgr = yr[:, :, sym, sc]  # [B, A, P]
        sgi = yi[:, :, sym, sc]
        sgr = np.ascontiguousarray(
            sgr.transpose(0, 2, 1).reshape(B, P_EST // 128, 128, A)
        )
        sgi = np.ascontiguousarray(
            sgi.transpose(0, 2, 1).reshape(B, P_EST // 128, 128, A)
        )
        return [
            {
                "sgr": sgr[c * B_LOC : (c + 1) * B_LOC],
                "sgi": sgi[c * B_LOC : (c + 1) * B_LOC],
                "wt": wtm,
            }
            for c in range(N_CORES)
        ]


_DD, _II = np.meshgrid(np.arange(ND), np.arange(A), indexing="ij")
_JJ = (_II + _DD) % A


def kernel(y_real, y_imag, estimation_indices, closest_subcarrier):
    from concourse.bass_utils import run_bass_kernel_spmd

    assert y_real.shape == (B, R, A, T, S), y_real.shape
    est = np.asarray(estimation_indices)
    closest = np.asarray(closest_subcarrier)
    kind, nc, extra = _get_program(est, closest)

    yr = np.ascontiguousarray(np.asarray(y_real, dtype=np.float32)[:, 0])
    yi = np.ascontiguousarray(np.asarray(y_imag, dtype=np.float32)[:, 0])
    in_maps = _make_in_maps(kind, extra, yr, yi, est)

    res = run_bass_kernel_spmd(nc, in_maps, list(range(N_CORES)))
    parts = [res.results[c]["out"] for c in range(N_CORES)]
    full = np.concatenate(parts, axis=0)

    if kind == "fast":
        # full: [B, 4, 128, 2*NV] fp16 -> band values v[b, s', d, i]
        full = full.reshape(B, S2, 2, ND, A).astype(np.float32)
        v = (full[:, :, 0] + 1j * full[:, :, 1]).astype(np.complex64)
        cov_half = np.empty((B, S2, A, A), dtype=np.complex64)
        cov_half[:, :, _II, _JJ] = v
        cov_half[:, :, _JJ, _II] = np.conj(v)
        cov = np.repeat(cov_half, 2, axis=1)  # NN expand to all S
    else:
        # full: [B, S, AA2] interleaved (ij, ri)
        cov = full.view(np.complex64).reshape(B, S, A, A)

    out = np.broadcast_to(
        cov.reshape(B, 1, 1, S, A, A), (B, R, T, S, A, A)
    )
    return np.ascontiguousarray(out)
